# revision 42
# baseline (speedup 1.0000x reference)
"""Bahdanau attention Trainium2 kernel.

Reference computation (per batch b, head h):
    f_proj = features[b] @ W1[h] + b1[h]            # [L, U]
    h_proj = hidden[b,h] @ W2[h] + b2[h]            # [U]
    act    = tanh(f_proj + h_proj)                  # [L, U]
    score  = act @ V[h] (+ bV[h], softmax-invariant)# [L]
    attn   = softmax_L(score)
    ctx    = attn @ features[b]                     # [D]
Outputs: context [B, H, D], attention_weights [B, L, H, 1].

Sharding: data-parallel over batch. 8 cores x 2 batches each; every core
computes all 8 heads for its batches, no collectives.

Device dataflow per core (B_LOC=2 batches), bf16 matmuls / f32 accumulate:
  phase 1: h_proj for all (h, b) on PE with K augmented by a (b1+b2) row ->
           bias vectors [U(part), (h,ut,b)] for the scalar-engine bias port.
           Emitted one head ahead of phase 2's first sweep (DMA overlap).
  phase 2: per (nb, L-chunk 512), for (ut, h): 4 K-tile matmuls W1^T @ fT
           -> PSUM -> scalar-engine tanh(psum + bias) -> bf16 act tile ->
           score matmul (M=2, V zero-padded stationary) accumulating into
           PE column-group h%4 via tile_position, one PSUM bank per group,
           so 4 score matmuls stream concurrently. Emitted in lagged
           4-bursts so the PE never waits on the scalar engine.
           Per chunk: online-softmax stage (local max / exp / local sum)
           runs on DVE+ACT underneath the PE stream.
  phase 3 (per nb): softmax combine: global max, rescale, 1/sum.
  phase 4: transpose+compact attn -> [L(part), head] via one fp32 matmul
           per L-tile against a 0/1 selection matrix (exact).
  phase 5: ctx matmuls (attn^T stationary, features moving), DMA out.
Head axis leaves the device in col-group order; the host undoes it.
"""

import sys

import numpy as np

if "/opt/trn_rl_repo" not in sys.path:
    sys.path.insert(0, "/opt/trn_rl_repo")

import ml_dtypes

B, L, D, H, U = 16, 2048, 512, 8, 512
N_CORES = 8
B_LOC = B // N_CORES  # batches per core
KT = D // 128  # K tiles of the feature/hidden contraction
UT = U // 128  # M tiles of the unit axis
LT = L // 128
LC = 4  # L chunks of 512 in phase 2
LCS = L // LC

BF16 = ml_dtypes.bfloat16

_CACHE = {}


def _build_bass():
    import concourse.bacc as bacc
    import concourse.mybir as mybir
    from concourse.tile import TileContext

    dt = mybir.dt
    AF = mybir.ActivationFunctionType
    ALU = mybir.AluOpType
    AX = mybir.AxisListType

    # Bacc (not plain Bass): its compile() pipeline runs
    # generate_event_semaphores, which splits multi-wait instructions down
    # to the 1-wait-per-instruction TRN2 ISA limit.
    nc = bacc.Bacc(
        "TRN2",
        target_bir_lowering=False,
        debug=False,
        num_devices=N_CORES,
    )

    ft_d = nc.dram_tensor("ft", [B_LOC, KT, 128, L], dt.bfloat16, kind="ExternalInput")
    feat_d = nc.dram_tensor("feat", [B_LOC, LT, 128, D], dt.bfloat16, kind="ExternalInput")
    w1_d = nc.dram_tensor("w1", [128, H, KT, U], dt.bfloat16, kind="ExternalInput")
    w2_d = nc.dram_tensor("w2", [128, H, KT, U], dt.bfloat16, kind="ExternalInput")
    b12_d = nc.dram_tensor("b12", [1, H, U], dt.bfloat16, kind="ExternalInput")
    ht_d = nc.dram_tensor("ht", [128, KT, B_LOC, H], dt.bfloat16, kind="ExternalInput")
    vblk_d = nc.dram_tensor("vblk", [128, H, UT, 2], dt.bfloat16, kind="ExternalInput")
    sel_d = nc.dram_tensor("sel", [128, H], dt.float32, kind="ExternalInput")

    ctx_d = nc.dram_tensor("ctx_out", [B_LOC, H, D], dt.float32, kind="ExternalOutput")
    attn_d = nc.dram_tensor("attn_out", [B_LOC, L, H], dt.float32, kind="ExternalOutput")

    with TileContext(nc) as tc, tc.tile_pool(name="const", bufs=1) as const:
        # ---- resident SBUF tensors --------------------------------------
        w1_sb = const.tile([128, H, KT, U], dt.bfloat16, name="w1_sb")
        w2_sb = const.tile([128, H, KT, U], dt.bfloat16, name="w2_sb")
        b12_sb = const.tile([1, H, U], dt.bfloat16, name="b12_sb")
        ht_sb = const.tile([128, KT, B_LOC, H], dt.bfloat16, name="ht_sb")
        vblk_sb = const.tile([128, H, UT, 2], dt.bfloat16, name="vblk_sb")
        sel_sb = const.tile([128, H], dt.float32, name="sel_sb")
        ones_sb = const.tile([1, B_LOC], dt.bfloat16, name="ones_sb")
        warm_sb = const.tile([128, 640], dt.bfloat16, name="warm_sb")
        wsink_sb = const.tile([1, 1], dt.float32, name="wsink_sb")
        ft_sb = const.tile([128, B_LOC, KT, L], dt.bfloat16, name="ft_sb")
        feat_sb = const.tile([128, B_LOC, LT, D], dt.bfloat16, name="feat_sb")
        bias_sb = const.tile([128, H * UT * B_LOC], dt.float32, name="bias_sb")

        # ---- input DMAs, ordered by first use ---------------------------
        nc.sync.dma_start(ht_sb[:], ht_d[:])
        nc.sync.dma_start(b12_sb[:], b12_d[:])
        nc.sync.dma_start(vblk_sb[:], vblk_d[:])
        nc.sync.dma_start(sel_sb[:], sel_d[:])
        nc.gpsimd.memset(ones_sb[:], 1.0)
        nc.gpsimd.memset(warm_sb[:], 0.0)
        for kt in range(KT):
            nc.sync.dma_start(w2_sb[:, 0, kt], w2_d[:, 0, kt])
        # first L-chunk of batch 0 (all K tiles) so main matmuls start early
        for kt in range(KT):
            nc.sync.dma_start(ft_sb[:, 0, kt, 0:LCS], ft_d[0, kt, :, 0:LCS])
        for h in range(H):
            nc.sync.dma_start(w1_sb[:, h], w1_d[:, h])
            if h + 1 < H:
                nc.sync.dma_start(w2_sb[:, h + 1], w2_d[:, h + 1])
        for kt in range(KT):
            nc.sync.dma_start(ft_sb[:, 0, kt, LCS:], ft_d[0, kt, :, LCS:])
        if B_LOC > 1:
            for nb in range(1, B_LOC):
                for kt in range(KT):
                    nc.sync.dma_start(ft_sb[:, nb, kt], ft_d[nb, kt])
        for nb in range(B_LOC):
            nc.sync.dma_start(
                feat_sb[:, nb], feat_d[nb].rearrange("lt p d -> p lt d")
            )

        # ---- PSUM pools --------------------------------------------------
        with (
            tc.tile_pool(name="pmain", bufs=3, space="PSUM") as pmain,
            tc.tile_pool(name="pscore", bufs=1, space="PSUM") as pscore,
            tc.tile_pool(name="pmisc", bufs=1, space="PSUM") as pmisc,
            tc.tile_pool(name="acts", bufs=6) as acts,
            tc.tile_pool(name="soft", bufs=2) as soft,
        ):
            # ---- phase 0: PE warmup on memset junk during the initial DMA
            # wait. The HAM clock gate keeps the PE at 1.2 GHz until it has
            # seen ~3.4 us of sustained activity; these matmuls depend only
            # on a memset, so they fill the otherwise-idle ramp and the real
            # work below starts at 2.4 GHz.
            wp = pmain.tile([128, LCS], dt.float32, tag="mp", name="wp")
            for _ in range(12):
                nc.tensor.matmul(
                    wp[:],
                    lhsT=warm_sb[:, 0:128],
                    rhs=warm_sb[:, 128:640],
                    start=True,
                    stop=True,
                )
            # read the sink so liveness never prunes the warmup chain
            nc.vector.tensor_copy(wsink_sb[:], wp[0:1, 0:1])

            # ---- phase 1: bias[(h,ut,b)] = W2[h]^T hidden[b,h] + b1 + b2
            # emitted one head ahead of the main loop's first sweep so the
            # PE starts as soon as the first DMA chunks land
            hp_ps = pmisc.tile([128, H * UT * B_LOC], dt.float32, tag="misc")

            def emit_hproj(h):
                for ut in range(UT):
                    o = hp_ps[:, (h * UT + ut) * B_LOC : (h * UT + ut + 1) * B_LOC]
                    for kt in range(KT):
                        nc.tensor.matmul(
                            o,
                            lhsT=w2_sb[:, h, kt, ut * 128 : (ut + 1) * 128],
                            rhs=ht_sb[:, kt, :, h],
                            start=(kt == 0),
                            stop=False,
                        )
                    nc.tensor.matmul(
                        o,
                        lhsT=b12_sb[0:1, h, ut * 128 : (ut + 1) * 128],
                        rhs=ones_sb[0:1, :],
                        start=False,
                        stop=True,
                    )
                nc.vector.tensor_copy(
                    bias_sb[:, h * UT * B_LOC : (h + 1) * UT * B_LOC],
                    hp_ps[:, h * UT * B_LOC : (h + 1) * UT * B_LOC],
                )

            emit_hproj(0)

            # ---- phases 2-5, pipelined per local batch ----------------
            # score_sb rows are in "grouped" order r = 2*(h%4) + h//4: head h
            # lands in PE column-group h%4 (tile_position), so its score row
            # sits at PSUM partition 32*(h%4) + h//4. Downstream APs undo the
            # permutation with a (j k)->(k j) rearrange.
            NP = 98  # score rows live sparse at partitions 32*j + k, j<4, k<2
            for nb in range(B_LOC):
                score_sb = soft.tile([128, L], dt.float32, tag="score")
                # unwritten rows must hold finite junk (0): the selection
                # matmul later multiplies them by 0.0, and 0*NaN = NaN.
                nc.gpsimd.memset(score_sb[:], 0.0)
                exp_sb = soft.tile([128, L], dt.float32, tag="exp")
                lmax = soft.tile([128, LC], dt.float32, tag="lmax")
                nlm = soft.tile([128, LC], dt.float32, tag="nlm")
                lsum = soft.tile([128, LC], dt.float32, tag="lsum")
                for lc in range(LC):
                    # one PSUM bank per col-group so each group runs a normal
                    # start/stop accumulation chain in its own bank
                    sc = [
                        pscore.tile(
                            [128, LCS], dt.float32, tag=f"sc{j}", name=f"sc{j}"
                        )
                        for j in range(4)
                    ]
                    pending = []
                    for ut in range(UT):
                        for h in range(H):
                            mp = pmain.tile([128, LCS], dt.float32, tag="mp")
                            for kt in range(KT):
                                nc.tensor.matmul(
                                    mp[:],
                                    lhsT=w1_sb[:, h, kt, ut * 128 : (ut + 1) * 128],
                                    rhs=ft_sb[:, nb, kt, lc * LCS : (lc + 1) * LCS],
                                    start=(kt == 0),
                                    stop=(kt == KT - 1),
                                )
                            if nb == 0 and lc == 0 and ut == 0 and h + 1 < H:
                                emit_hproj(h + 1)
                            if len(pending) == 4:
                                # burst of 4 score matmuls, one per col-group:
                                # they stream concurrently through the PE
                                for kw in pending:
                                    nc.tensor.matmul(**kw)
                                pending.clear()
                            at = acts.tile([128, LCS], dt.bfloat16, tag="at")
                            bidx = (h * UT + ut) * B_LOC + nb
                            nc.scalar.activation(
                                at[:],
                                mp[:],
                                AF.Tanh,
                                bias=bias_sb[:, bidx : bidx + 1],
                            )
                            j = h % 4
                            pending.append(
                                dict(
                                    out=sc[j][32 * j : 32 * j + 2, :],
                                    lhsT=vblk_sb[:, h, ut, :],
                                    rhs=at[:],
                                    start=(ut == 0 and h < 4),
                                    stop=(ut == UT - 1 and h >= 4),
                                    tile_position=(0, 32 * j),
                                )
                            )
                    for kw in pending:
                        nc.tensor.matmul(**kw)
                    pending.clear()
                    for j in range(4):
                        nc.vector.tensor_copy(
                            score_sb[32 * j : 32 * j + 2, lc * LCS : (lc + 1) * LCS],
                            sc[j][32 * j : 32 * j + 2, :],
                        )
                    # online softmax, chunk stage (overlaps phase-2 PE work):
                    # local max, exp(score - lmax), local sum
                    ch = slice(lc * LCS, (lc + 1) * LCS)
                    nc.vector.tensor_reduce(
                        lmax[0:NP, lc : lc + 1],
                        score_sb[0:NP, ch],
                        axis=AX.X,
                        op=ALU.max,
                    )
                    nc.vector.tensor_scalar_mul(
                        nlm[0:NP, lc : lc + 1], lmax[0:NP, lc : lc + 1], -1.0
                    )
                    nc.scalar.activation(
                        exp_sb[0:NP, ch],
                        score_sb[0:NP, ch],
                        AF.Exp,
                        bias=nlm[0:NP, lc : lc + 1],
                    )
                    nc.vector.tensor_reduce(
                        lsum[0:NP, lc : lc + 1],
                        exp_sb[0:NP, ch],
                        axis=AX.X,
                        op=ALU.add,
                    )

                # ---- phase 3: softmax combine (short tail) --------------
                gmx = soft.tile([128, 1], dt.float32, tag="gmx")
                nc.vector.tensor_reduce(
                    gmx[0:NP], lmax[0:NP], axis=AX.X, op=ALU.max
                )
                ngmx = soft.tile([128, 1], dt.float32, tag="ngmx")
                nc.vector.tensor_scalar_mul(ngmx[0:NP], gmx[0:NP], -1.0)
                scl = soft.tile([128, LC], dt.float32, tag="scl")
                nc.scalar.activation(
                    scl[0:NP], lmax[0:NP], AF.Exp, bias=ngmx[0:NP]
                )
                ssum = soft.tile([128, LC], dt.float32, tag="ssum")
                nc.vector.tensor_mul(ssum[0:NP], lsum[0:NP], scl[0:NP])
                gsum = soft.tile([128, 1], dt.float32, tag="gsum")
                nc.vector.tensor_reduce(
                    gsum[0:NP], ssum[0:NP], axis=AX.X, op=ALU.add
                )
                rc = soft.tile([128, 1], dt.float32, tag="rc")
                nc.vector.reciprocal(rc[0:NP], gsum[0:NP])
                attn_sb = soft.tile([128, L], dt.float32, tag="attn")
                for lc in range(LC):
                    ch = slice(lc * LCS, (lc + 1) * LCS)
                    nc.vector.tensor_scalar(
                        attn_sb[0:NP, ch],
                        exp_sb[0:NP, ch],
                        scl[0:NP, lc : lc + 1],
                        rc[0:NP, 0:1],
                        op0=ALU.mult,
                        op1=ALU.mult,
                    )

                # ---- phase 4: transpose+compact attn -> [L(part), r] ----
                # regular fp32 matmul with a 0/1 selection matrix: exact
                # (each output element is a single 1.0*x product) and it
                # compacts the sparse rows to dense grouped columns.
                tp = pmisc.tile([128, LT * H], dt.float32, tag="misc")
                for lt in range(LT):
                    nc.tensor.matmul(
                        tp[:, lt * H : (lt + 1) * H],
                        lhsT=attn_sb[0:NP, lt * 128 : (lt + 1) * 128],
                        rhs=sel_sb[0:NP, :],
                        start=True,
                        stop=True,
                    )
                attnT_sb = soft.tile([128, LT, H], dt.float32, tag="attnT")
                nc.vector.tensor_copy(
                    attnT_sb[:].rearrange("p lt h -> p (lt h)"), tp[:]
                )
                attnT_bf = soft.tile([128, LT, H], dt.bfloat16, tag="attnTb")
                nc.vector.tensor_copy(
                    attnT_bf[:].rearrange("p lt h -> p (lt h)"), tp[:]
                )
                # head axis stays in grouped order; host permutes after gather
                nc.sync.dma_start(
                    attn_d[nb].rearrange("(lt p) h -> p lt h", p=128), attnT_sb[:]
                )

                # ---- phase 5: ctx = attn @ features ---------------------
                cp = pmisc.tile([H, D], dt.float32, tag="misc")
                for lt in range(LT):
                    nc.tensor.matmul(
                        cp[:],
                        lhsT=attnT_bf[:, lt, :],
                        rhs=feat_sb[:, nb, lt, :],
                        start=(lt == 0),
                        stop=(lt == LT - 1),
                    )
                ctx_sb = soft.tile([H, D], dt.float32, tag="ctx")
                nc.vector.tensor_copy(ctx_sb[:], cp[:])
                nc.sync.dma_start(ctx_d[nb], ctx_sb[:])

    nc.compile()
    return nc


def _prep_inputs(features, hidden, W1, b1, W2, b2, V):
    """Host-side layout prep. Returns per-core input maps."""
    # features transposed [b, kt, p(d), l] and natural [b, lt, p(l), d]
    ft = np.ascontiguousarray(features.transpose(0, 2, 1)).reshape(B, KT, 128, L)
    ft = ft.astype(BF16)
    feat = features.reshape(B, LT, 128, D).astype(BF16)
    # weights [p(d within kt), h, kt, u]
    w1 = np.ascontiguousarray(
        W1.reshape(H, KT, 128, U).transpose(2, 0, 1, 3)
    ).astype(BF16)
    w2 = np.ascontiguousarray(
        W2.reshape(H, KT, 128, U).transpose(2, 0, 1, 3)
    ).astype(BF16)
    b12 = (b1 + b2).reshape(1, H, U).astype(BF16)
    # hidden transposed [p(d), kt, b, h] (sliced per core below)
    ht = np.ascontiguousarray(
        hidden.transpose(2, 0, 1).reshape(KT, 128, B, H).transpose(1, 0, 2, 3)
    ).astype(BF16)
    # per-head V stationary for col-group h%4: two columns (the group's two
    # heads); head h occupies column h//4, the other column is zero.
    vblk = np.zeros((128, H, UT, 2), dtype=BF16)
    vt = V.reshape(H, UT, 128).astype(BF16)
    for h in range(H):
        vblk[:, h, :, h // 4] = vt[h].T
    # selection matrix: sel[32*j + k, 2*j + k] = 1 compacts the sparse
    # score/attn rows (partition 32j+k) into dense grouped columns.
    sel = np.zeros((128, H), dtype=np.float32)
    for j in range(4):
        for k in range(2):
            sel[32 * j + k, 2 * j + k] = 1.0

    in_maps = []
    for c in range(N_CORES):
        b0 = c * B_LOC
        in_maps.append(
            {
                "ft": ft[b0 : b0 + B_LOC],
                "feat": feat[b0 : b0 + B_LOC],
                "w1": w1,
                "w2": w2,
                "b12": b12,
                "ht": np.ascontiguousarray(ht[:, :, b0 : b0 + B_LOC]),
                "vblk": vblk,
                "sel": sel,
            }
        )
    return in_maps


def run(inputs, trace=False, trace_kwargs=None):
    """Run on hardware; returns (context, attention_weights, BassKernelResults)."""
    from concourse.bass_utils import run_bass_kernel_spmd

    if "nc" not in _CACHE:
        _CACHE["nc"] = _build_bass()
    nc = _CACHE["nc"]

    in_maps = _prep_inputs(
        np.asarray(inputs["features"], np.float32),
        np.asarray(inputs["hidden"], np.float32),
        np.asarray(inputs["W1"], np.float32),
        np.asarray(inputs["b1"], np.float32),
        np.asarray(inputs["W2"], np.float32),
        np.asarray(inputs["b2"], np.float32),
        np.asarray(inputs["V"], np.float32),
    )
    res = run_bass_kernel_spmd(
        nc,
        in_maps,
        list(range(N_CORES)),
        trace=trace,
        **(trace_kwargs or {}),
    )
    # device head axis is in col-group order r = 2*(h%4) + h//4; undo it
    perm = np.array([2 * (h % 4) + h // 4 for h in range(H)])
    context = np.concatenate(
        [res.results[c]["ctx_out"] for c in range(N_CORES)], axis=0
    ).astype(np.float32)[:, perm, :]
    attn = np.concatenate(
        [res.results[c]["attn_out"] for c in range(N_CORES)], axis=0
    ).astype(np.float32)[:, :, perm, None]
    return context, attn, res


def kernel(**inputs):
    context, attn, _ = run(inputs, trace=False)
    return context, attn
